# revision 25
# baseline (speedup 1.0000x reference)
"""Trainium2 Bass kernel for nn_AugmentedAttentionHead.

See kernel design notes: data-parallel over batch (8/core); transposed
softmax ([k-part, q-free]) so exp() output feeds attn@v directly as the
stationary operand; log-space Gaussian bias reconstructed by one selector
matmul per k-chunk; x host-pre-transposed; f32r matmuls; ln/exp-only ACT
table; prefix kills folded into host tables.

The per-batch work is split into 5 phases and traced along anti-diagonals
(software pipelining across batches) so each engine always has another
batch's independent work to fill dependency stalls.
"""

import numpy as np
from contextlib import ExitStack

B, T, E, H = 64, 577, 768, 64
GRID = 24
EPS = 1e-5
NCORES = 8
BPC = B // NCORES

TSZ = [128, 128, 128, 128, 65]
TOFF = [0, 128, 256, 384, 512]
NCH = 5
FW = 50
NEG = -1.0e30
NPH = 5


def _host_consts(w_q, w_k, w_v, w_sigma, w_alpha):
    w_ext = np.zeros((6, 128, 256), np.float32)
    wqkv = np.concatenate([w_q, w_k, w_v], axis=1)
    for j in range(6):
        w_ext[j, :, :192] = wqkv[j * 128:(j + 1) * 128]

    w_sa = np.zeros((64, 4), np.float32)
    w_sa[:, 0:2] = 8.0 * w_sigma
    w_sa[:, 2] = 8.0 * w_alpha[:, 0]

    dxy2s = np.zeros((128, NCH * FW), np.float32)
    for c in range(NCH):
        for r in range(TSZ[c]):
            t = TOFF[c] + r
            o = c * FW
            dxy2s[r, o + 49] = NEG
            if t == 0:
                dxy2s[r, o:o + 24] = NEG
                dxy2s[r, o + 25:o + 49] = NEG
                continue
            p = t - 1
            py, px = p // GRID, p % GRID
            j = np.arange(GRID, dtype=np.float32)
            dxy2s[r, o:o + 24] = -0.5 * (py - j) ** 2
            dxy2s[r, o + 25:o + 49] = -0.5 * (px - j) ** 2

    sel = np.zeros((FW, NCH * 128), np.float32)
    for c in range(NCH):
        for r in range(TSZ[c]):
            t = TOFF[c] + r
            if t == 0:
                sel[49, c * 128 + r] = 1.0
                continue
            p = t - 1
            sel[p // GRID, c * 128 + r] = 1.0
            sel[24, c * 128 + r] = 1.0
            sel[25 + p % GRID, c * 128 + r] = 1.0

    ident = np.eye(128, dtype=np.float32)
    onz = np.stack([np.ones(128), np.zeros(128)], 1).astype(np.float32)
    return w_ext, w_sa, dxy2s, sel, ident, onz


def _trace(nc, tc, ctx, consts_f, need_gb):
    import concourse.mybir as mybir

    dt = mybir.dt
    AF = mybir.ActivationFunctionType
    OP = mybir.AluOpType
    bs0, bs1, ba0 = consts_f[:3]

    xT_d = nc.dram_tensor("xT", [BPC, E, T], dt.float32r, kind="ExternalInput").ap()
    wext_d = nc.dram_tensor("w_ext", [6, 128, 256], dt.float32r, kind="ExternalInput").ap()
    wsa_d = nc.dram_tensor("w_sa", [64, 4], dt.float32r, kind="ExternalInput").ap()
    dxy_d = nc.dram_tensor("dxy2s", [128, NCH * FW], dt.float32, kind="ExternalInput").ap()
    sel_d = nc.dram_tensor("sel", [FW, NCH * 128], dt.float32r, kind="ExternalInput").ap()
    id_d = nc.dram_tensor("ident", [128, 128], dt.float32r, kind="ExternalInput").ap()
    onz_d = nc.dram_tensor("onz", [128, 2], dt.float32r, kind="ExternalInput").ap()
    if need_gb:
        gb_d = nc.dram_tensor("gb", [4, 64], dt.float32, kind="ExternalInput").ap()
    out_d = nc.dram_tensor("out", [BPC, T, H], dt.float32, kind="ExternalOutput").ap()

    cpool = ctx.enter_context(tc.tile_pool(name="consts", bufs=1))
    xpool = ctx.enter_context(tc.tile_pool(name="x", bufs=18))
    wkpool = ctx.enter_context(tc.tile_pool(name="work", bufs=4))
    rpool = ctx.enter_context(tc.tile_pool(name="raw", bufs=10))
    qkpool = ctx.enter_context(tc.tile_pool(name="qk", bufs=10))
    vpool = ctx.enter_context(tc.tile_pool(name="v", bufs=25))
    apool = ctx.enter_context(tc.tile_pool(name="attn", bufs=12))
    spool = ctx.enter_context(tc.tile_pool(name="small", bufs=4))
    opool = ctx.enter_context(tc.tile_pool(name="outb", bufs=3))

    # PSUM (8 banks): {qkv, transpose, sigma-alpha} x2 + z/L x6 (1-bank slots)
    ps_sm = ctx.enter_context(tc.tile_pool(name="ps_sm", bufs=2, space="PSUM"))
    ps_zl = ctx.enter_context(tc.tile_pool(name="ps_zl", bufs=3, space="PSUM"))

    w_ext = cpool.tile([128, 6 * 256], dt.float32r)
    for j in range(6):
        nc.sync.dma_start(w_ext[:, j * 256:(j + 1) * 256], wext_d[j])
    w_sa = cpool.tile([64, 4], dt.float32r)
    nc.sync.dma_start(w_sa[:], wsa_d)
    dxy2s = cpool.tile([128, NCH * FW], dt.float32)
    nc.sync.dma_start(dxy2s[:], dxy_d)
    sel = cpool.tile([FW, NCH * 128], dt.float32r)
    nc.sync.dma_start(sel[:], sel_d)
    ident = cpool.tile([128, 128], dt.float32r)
    nc.sync.dma_start(ident[:], id_d)
    onz = cpool.tile([128, 2], dt.float32r)
    nc.sync.dma_start(onz[:], onz_d)
    gb = None
    if need_gb:
        gb = cpool.tile([4, 64], dt.float32)
        nc.sync.dma_start(gb[:], gb_d)

    _bias_cache = {}

    def fbias(val, tsz=128):
        val = float(val)
        if val == 0.0:
            return 0.0
        if val not in _bias_cache:
            bt = cpool.tile([128, 1], dt.float32, name=f"bias{len(_bias_cache)}")
            nc.vector.memset(bt[:], val)
            _bias_cache[val] = bt
        return _bias_cache[val][0:tsz, :]

    def phase1(S, b):
        """x DMA, QKV matmuls, raw evac, LN stats."""
        S["xt"] = xt = []
        for j in range(6):
            xj = xpool.tile([128, T], dt.float32r, tag="xT", name=f"x{b}_{j}")
            nc.sync.dma_start(xj[:], xT_d[b, j * 128:(j + 1) * 128, :])
            xt.append(xj)
        S["raw"] = raw = [
            rpool.tile([128, 192], dt.float32, tag="raw", name=f"raw{b}_{i}")
            for i in range(NCH)]
        S["mv"] = mv = spool.tile([128, 20], dt.float32, tag="mv", name=f"mv{b}")
        for c in range(NCH):
            tsz, toff = TSZ[c], TOFF[c]
            pqkv = ps_sm.tile([128, 256], dt.float32, tag="sm", name=f"qkv{b}_{c}")
            for j in range(6):
                nc.tensor.matmul(
                    pqkv[0:tsz, :], xt[j][:, toff:toff + tsz],
                    w_ext[:, j * 256:(j + 1) * 256],
                    start=(j == 0), stop=(j == 5),
                )
            nc.vector.tensor_copy(raw[c][0:tsz, :], pqkv[0:tsz, 0:192])
            st = spool.tile([128, 12], dt.float32, tag="st", name=f"st{b}_{c}")
            nc.vector.bn_stats(st[0:tsz, 0:6], raw[c][0:tsz, 0:64])
            nc.vector.bn_stats(st[0:tsz, 6:12], raw[c][0:tsz, 64:128])
            nc.vector.bn_aggr(mv[0:tsz, 4 * c:4 * c + 2], st[0:tsz, 0:6])
            nc.vector.bn_aggr(mv[0:tsz, 4 * c + 2:4 * c + 4], st[0:tsz, 6:12])

    def phase2(S, b):
        """LN params + apply, v build, q/k transposes into qkT."""
        mv = S["mv"]
        raw = S["raw"]
        mv4 = mv[:].rearrange("p (c f) -> p c f", f=4)
        lnv = spool.tile([128, 10], dt.float32, tag="lnv", name=f"lnv{b}")
        lnv2 = lnv[:].rearrange("p (c f) -> p c f", f=2)
        nc.scalar.activation(lnv2[:, :, :], mv4[:, :, 1::2], AF.Ln, bias=fbias(EPS))
        sc = spool.tile([128, 10], dt.float32, tag="sc", name=f"sc{b}")
        sc2 = sc[:].rearrange("p (c f) -> p c f", f=2)
        nc.scalar.activation(sc2[:, :, 0], lnv2[:, :, 0], AF.Exp,
                             bias=fbias(float(np.log(0.125))), scale=-0.5)
        nc.scalar.activation(sc2[:, :, 1], lnv2[:, :, 1], AF.Exp, scale=-0.5)
        nmr = spool.tile([128, 10], dt.float32, tag="nmr", name=f"nmr{b}")
        nmr2 = nmr[:].rearrange("p (c f) -> p c f", f=2)
        nc.vector.scalar_tensor_tensor(
            nmr2[:, :, :], mv4[:, :, 0::2], -1.0, sc2[:, :, :], OP.mult, OP.mult)

        S["qkT"] = qkT = wkpool.tile([64, 1156], dt.float32r, tag="qkT",
                                     name=f"qkT{b}")
        S["v_ext"] = v_ext = [
            vpool.tile([128, 66], dt.bfloat16, tag="vext", name=f"vext{b}_{i}")
            for i in range(NCH)]
        for c in range(NCH):
            tsz, toff = TSZ[c], TOFF[c]
            t2 = tsz + (tsz & 1)
            qk_ln = qkpool.tile([128, 128], dt.float32r, tag="qkln",
                                name=f"qkln{b}_{c}")
            if t2 != tsz:
                nc.vector.tensor_copy(
                    qk_ln[64:66, :], onz[64:66, 1:2].broadcast_to([2, 128]))
            nc.vector.tensor_scalar(
                qk_ln[0:tsz, 0:64], raw[c][0:tsz, 0:64],
                sc[0:tsz, 2 * c:2 * c + 1], nmr[0:tsz, 2 * c:2 * c + 1],
                OP.mult, OP.add)
            nc.vector.tensor_scalar(
                qk_ln[0:tsz, 64:128], raw[c][0:tsz, 64:128],
                sc[0:tsz, 2 * c + 1:2 * c + 2], nmr[0:tsz, 2 * c + 1:2 * c + 2],
                OP.mult, OP.add)
            if need_gb:
                nc.vector.tensor_mul(qk_ln[0:tsz, 0:64], qk_ln[0:tsz, 0:64],
                                     gb[0:1, :].partition_broadcast(tsz))
                nc.vector.tensor_add(qk_ln[0:tsz, 0:64], qk_ln[0:tsz, 0:64],
                                     gb[1:2, :].partition_broadcast(tsz))
                nc.vector.tensor_mul(qk_ln[0:tsz, 64:128], qk_ln[0:tsz, 64:128],
                                     gb[2:3, :].partition_broadcast(tsz))
                nc.vector.tensor_add(qk_ln[0:tsz, 64:128], qk_ln[0:tsz, 64:128],
                                     gb[3:4, :].partition_broadcast(tsz))
            nc.vector.tensor_scalar(
                v_ext[c][0:tsz, 0:64], raw[c][0:tsz, 128:192], 1.0, None, OP.mult)
            nc.vector.tensor_copy(v_ext[c][0:tsz, 64:66], onz[0:tsz, :])

            tpc = ps_sm.tile([128, 256], dt.float32r, tag="sm", name=f"tp{b}_{c}")
            nc.tensor.transpose(tpc[0:64, 0:t2], qk_ln[0:t2, 0:64],
                                ident[0:t2, 0:t2])
            nc.tensor.transpose(tpc[0:64, 128:128 + t2], qk_ln[0:t2, 64:128],
                                ident[0:t2, 0:t2])
            ncols = min(128, 578 - toff)
            dst = qkT[:].rearrange("p (g q) -> p g q", g=2)[:, :, toff:toff + ncols]
            nc.vector.tensor_copy(
                dst, tpc[0:64, :].rearrange("p (g q) -> p g q", g=2)[:, :, 0:ncols])

    def phase3(S, b):
        """sigma/alpha MLP, log-space factor build, factor transposes."""
        qkT = S["qkT"]
        psa = ps_sm.tile([128, 256], dt.float32, tag="sm", name=f"psa{b}")
        for c in range(NCH):
            nc.tensor.matmul(
                psa[0:TSZ[c], 4 * c:4 * c + 4], qkT[:, TOFF[c]:TOFF[c] + TSZ[c]],
                w_sa[:], start=True, stop=True)
        sap = spool.tile([128, 20], dt.float32, tag="sap", name=f"sap{b}")
        nc.vector.tensor_copy(sap[:], psa[:, 0:20])
        sap4 = sap[:].rearrange("p (c f) -> p c f", f=4)
        texp = spool.tile([128, 10], dt.float32, tag="texp", name=f"texp{b}")
        texp3 = texp[:].rearrange("p (c f) -> p c f", f=2)
        if bs0 == bs1:
            nc.scalar.activation(texp3[:, :, :], sap4[:, :, 0:2], AF.Exp,
                                 bias=fbias(-bs0), scale=-1.0)
        else:
            for col in range(2):
                nc.scalar.activation(texp3[:, :, col], sap4[:, :, col], AF.Exp,
                                     bias=fbias(-(bs0 if col == 0 else bs1)),
                                     scale=-1.0)
        ab = spool.tile([128, 10], dt.float32, tag="ab", name=f"ab{b}")
        nc.vector.tensor_scalar_add(ab[:], texp[:], 1.0)
        nc.vector.tensor_mul(ab[:], ab[:], ab[:])
        spe = spool.tile([128, 5], dt.float32, tag="spe", name=f"spe{b}")
        nc.scalar.activation(spe[:], sap4[:, :, 2], AF.Exp, bias=fbias(ba0))
        spl = spool.tile([128, 5], dt.float32, tag="spl", name=f"spl{b}")
        nc.scalar.activation(spl[:], spe[:], AF.Ln, bias=fbias(1.0))
        lna = spool.tile([128, 5], dt.float32, tag="lna", name=f"lna{b}")
        nc.scalar.activation(lna[:], spl[:], AF.Ln, scale=0.125)

        yn = spool.tile([128, NCH * FW], dt.float32r, tag="yn", name=f"yn{b}")
        dxy3 = dxy2s[:].rearrange("p (c f) -> p c f", f=FW)
        yn3 = yn[:].rearrange("p (c f) -> p c f", f=FW)
        ab3 = ab[:].rearrange("p (c f) -> p c f", f=2)
        nc.vector.scalar_tensor_tensor(
            yn3[:, :, 0:24], dxy3[:, :, 0:24], 1.0,
            ab3[:, :, 1:2].broadcast_to([128, NCH, 24]), OP.mult, OP.mult)
        nc.vector.scalar_tensor_tensor(
            yn3[:, :, 25:50], dxy3[:, :, 25:50], 1.0,
            ab3[:, :, 0:1].broadcast_to([128, NCH, 25]), OP.mult, OP.mult)
        nc.vector.tensor_copy(yn3[:, :, 24:25], lna[:].unsqueeze(-1))

        S["yT"] = yT = wkpool.tile([FW, 578], dt.float32r, tag="yT", name=f"yT{b}")
        for c in range(NCH):
            t2 = TSZ[c] + (TSZ[c] & 1)
            pfc = ps_sm.tile([128, 256], dt.float32r, tag="sm", name=f"pf{b}_{c}")
            nc.tensor.transpose(pfc[0:FW, 0:t2], yn[0:t2, c * FW:(c + 1) * FW],
                                ident[0:t2, 0:t2])
            nc.vector.tensor_copy(yT[:, TOFF[c]:TOFF[c] + t2], pfc[0:FW, 0:t2])

    def phase4(S, b):
        """main loop: sim^T + bias + exp per k-chunk (transposed softmax)."""
        qkT, yT = S["qkT"], S["yT"]
        S["attnT"] = attnT = [
            apool.tile([128, T], dt.bfloat16, tag="attnT", name=f"attnT{b}_{i}")
            for i in range(NCH)]
        for c in range(NCH):
            tsz, toff = TSZ[c], TOFF[c]
            selc = sel[:, c * 128:c * 128 + tsz]
            kTc = qkT[:, 578 + toff:578 + toff + tsz]
            expl = wkpool.tile([128, 578], dt.float32r, tag="expl",
                               name=f"expl{b}_{c}")
            pl = ps_zl.tile([128, 640], dt.float32, tag="zl", name=f"pl{b}{c}")
            nc.tensor.matmul(pl[0:tsz, 0:512], selc, yT[:, 0:512],
                             start=True, stop=True)
            nc.tensor.matmul(pl[0:tsz, 512:578], selc, yT[:, 512:578],
                             start=True, stop=True)
            nc.scalar.activation(expl[0:tsz, :], pl[0:tsz, 0:578], AF.Exp)
            pz = ps_zl.tile([128, 640], dt.float32, tag="zl", name=f"pz{b}{c}")
            nc.tensor.matmul(pz[0:tsz, 0:512], kTc, qkT[:, 0:512],
                             start=True, stop=False)
            nc.tensor.matmul(pz[0:tsz, 512:578], kTc, qkT[:, 512:578],
                             start=True, stop=False)
            nc.tensor.matmul(pz[0:tsz, 0:512], ident[0:tsz, 0:tsz],
                             expl[0:tsz, 0:512], start=False, stop=True)
            nc.tensor.matmul(pz[0:tsz, 512:578], ident[0:tsz, 0:tsz],
                             expl[0:tsz, 512:578], start=False, stop=True)
            nc.scalar.activation(attnT[c][0:tsz, :], pz[0:tsz, 0:T],
                                 AF.Exp)

    def phase5(S, b):
        """attn @ [v|1] + batched normalize + store."""
        attnT, v_ext = S["attnT"], S["v_ext"]
        osb = opool.tile([128, 320], dt.float32, tag="osb", name=f"osb{b}")
        for g, qcs in enumerate(((0, 1), (2, 3), (4,))):
            po = ps_zl.tile([128, 256], dt.float32, tag="zl", name=f"po{b}_{g}")
            for i, qc in enumerate(qcs):
                qsz, qoff = TSZ[qc], TOFF[qc]
                for kc in range(NCH):
                    nc.tensor.matmul(
                        po[0:qsz, 128 * i:128 * i + 66],
                        attnT[kc][0:TSZ[kc], qoff:qoff + qsz],
                        v_ext[kc][0:TSZ[kc], :], start=(kc == 0), stop=(kc == 4))
            n = len(qcs)
            po5 = po[:].rearrange("p (c f) -> p c f", f=128)[:, 0:n, :]
            rcp = spool.tile([128, 2], dt.float32, tag="rcp", name=f"rcp{b}_{g}")
            nc.vector.reciprocal(rcp[:, 0:n], po5[:, :, 64])
            osb3 = osb[:].rearrange("p (c f) -> p c f", f=64)[:, 2 * g:2 * g + n, :]
            nc.vector.scalar_tensor_tensor(
                osb3, po5[:, :, 0:64], 1.0,
                rcp[:, 0:n].unsqueeze(-1).broadcast_to([128, n, 64]),
                OP.mult, OP.mult)
        nc.sync.dma_start(
            out_d[b, 0:512, :].rearrange("(c p) h -> p c h", p=128),
            osb[:, 0:256].rearrange("p (c h) -> p c h", h=64))
        nc.sync.dma_start(out_d[b, 512:T, :], osb[0:65, 256:320])

    # anti-diagonal software pipeline across batches
    phases = [phase1, phase2, phase3, phase4, phase5]
    states = [dict() for _ in range(BPC)]
    for slot in range(BPC + NPH - 1):
        for p in range(NPH - 1, -1, -1):
            b = slot - p
            if 0 <= b < BPC:
                phases[p](states[b], b)


_CACHE = {}


def _build(consts_f, need_gb):
    import concourse.tile as tile
    from concourse import bacc

    key = (consts_f, need_gb)
    if key in _CACHE:
        return _CACHE[key]
    nc = bacc.Bacc("TRN2", target_bir_lowering=False, debug=False)
    with tile.TileContext(nc) as tc, ExitStack() as ctx:
        _trace(nc, tc, ctx, consts_f, need_gb)
    nc.finalize()
    _CACHE[key] = nc
    return nc


def kernel(x, w_q, w_k, w_v, q_gamma, q_beta, k_gamma, k_beta,
           w_sigma, b_sigma, w_alpha, b_alpha):
    from concourse import bass_utils

    x = np.asarray(x, np.float32)
    w_q, w_k, w_v = (np.asarray(a, np.float32) for a in (w_q, w_k, w_v))
    w_sigma = np.asarray(w_sigma, np.float32)
    w_alpha = np.asarray(w_alpha, np.float32)
    b_sigma = np.asarray(b_sigma, np.float32)
    b_alpha = np.asarray(b_alpha, np.float32)
    q_gamma, q_beta = np.asarray(q_gamma, np.float32), np.asarray(q_beta, np.float32)
    k_gamma, k_beta = np.asarray(k_gamma, np.float32), np.asarray(k_beta, np.float32)

    trivial_gb = (
        np.allclose(q_gamma, 1) and np.allclose(k_gamma, 1)
        and np.allclose(q_beta, 0) and np.allclose(k_beta, 0)
    )

    w_ext, w_sa, dxy2s, sel, ident, onz = _host_consts(
        w_q, w_k, w_v, w_sigma, w_alpha)
    consts_f = (float(b_sigma[0]), float(b_sigma[1]), float(b_alpha[0]))
    nc = _build(consts_f, not trivial_gb)

    xt = np.ascontiguousarray(x.reshape(NCORES, BPC, T, E).transpose(0, 1, 3, 2))

    base = {
        "w_ext": w_ext, "w_sa": w_sa, "dxy2s": dxy2s, "sel": sel, "ident": ident,
        "onz": onz,
    }
    if not trivial_gb:
        base["gb"] = np.stack(
            [q_gamma, q_beta / 8.0, k_gamma, k_beta]).astype(np.float32)
    in_maps = [{**base, "xT": xt[c]} for c in range(NCORES)]

    res = bass_utils.run_bass_kernel_spmd(nc, in_maps, core_ids=list(range(NCORES)))
    out = np.concatenate([res.results[c]["out"] for c in range(NCORES)], axis=0)
    return out.astype(np.float32)


# revision 26
# speedup vs baseline: 1.1072x; 1.1072x over previous
"""Trainium2 Bass kernel for nn_AugmentedAttentionHead.

See kernel design notes: data-parallel over batch (8/core); transposed
softmax ([k-part, q-free]) so exp() output feeds attn@v directly as the
stationary operand; log-space Gaussian bias reconstructed by one selector
matmul per k-chunk; x host-pre-transposed; f32r matmuls; ln/exp-only ACT
table; prefix kills folded into host tables.

The per-batch work is split into 5 phases and traced along anti-diagonals
(software pipelining across batches) so each engine always has another
batch's independent work to fill dependency stalls.
"""

import numpy as np
from contextlib import ExitStack

B, T, E, H = 64, 577, 768, 64
GRID = 24
EPS = 1e-5
NCORES = 8
BPC = B // NCORES

TSZ = [128, 128, 128, 128, 65]
TOFF = [0, 128, 256, 384, 512]
NCH = 5
FW = 50
NEG = -1.0e30
NPH = 5


def _host_consts(w_q, w_k, w_v, w_sigma, w_alpha):
    w_ext = np.zeros((6, 128, 256), np.float32)
    wqkv = np.concatenate([w_q, w_k, w_v], axis=1)
    for j in range(6):
        w_ext[j, :, :192] = wqkv[j * 128:(j + 1) * 128]

    w_sa = np.zeros((64, 4), np.float32)
    w_sa[:, 0:2] = 8.0 * w_sigma
    w_sa[:, 2] = 8.0 * w_alpha[:, 0]

    dxy2s = np.zeros((128, NCH * FW), np.float32)
    for c in range(NCH):
        for r in range(TSZ[c]):
            t = TOFF[c] + r
            o = c * FW
            dxy2s[r, o + 49] = NEG
            if t == 0:
                dxy2s[r, o:o + 24] = NEG
                dxy2s[r, o + 25:o + 49] = NEG
                continue
            p = t - 1
            py, px = p // GRID, p % GRID
            j = np.arange(GRID, dtype=np.float32)
            dxy2s[r, o:o + 24] = -0.5 * (py - j) ** 2
            dxy2s[r, o + 25:o + 49] = -0.5 * (px - j) ** 2

    sel = np.zeros((FW, NCH * 128), np.float32)
    for c in range(NCH):
        for r in range(TSZ[c]):
            t = TOFF[c] + r
            if t == 0:
                sel[49, c * 128 + r] = 1.0
                continue
            p = t - 1
            sel[p // GRID, c * 128 + r] = 1.0
            sel[24, c * 128 + r] = 1.0
            sel[25 + p % GRID, c * 128 + r] = 1.0

    ident = np.eye(128, dtype=np.float32)
    onz = np.stack([np.ones(128), np.zeros(128)], 1).astype(np.float32)
    return w_ext, w_sa, dxy2s, sel, ident, onz


def _trace(nc, tc, ctx, consts_f, need_gb):
    import concourse.mybir as mybir

    dt = mybir.dt
    AF = mybir.ActivationFunctionType
    OP = mybir.AluOpType
    bs0, bs1, ba0 = consts_f[:3]

    xT_d = nc.dram_tensor("xT", [BPC, E, T], dt.float32r, kind="ExternalInput").ap()
    wext_d = nc.dram_tensor("w_ext", [6, 128, 256], dt.float32r, kind="ExternalInput").ap()
    wsa_d = nc.dram_tensor("w_sa", [64, 4], dt.float32r, kind="ExternalInput").ap()
    dxy_d = nc.dram_tensor("dxy2s", [128, NCH * FW], dt.float32, kind="ExternalInput").ap()
    sel_d = nc.dram_tensor("sel", [FW, NCH * 128], dt.float32r, kind="ExternalInput").ap()
    id_d = nc.dram_tensor("ident", [128, 128], dt.float32r, kind="ExternalInput").ap()
    onz_d = nc.dram_tensor("onz", [128, 2], dt.float32r, kind="ExternalInput").ap()
    if need_gb:
        gb_d = nc.dram_tensor("gb", [4, 64], dt.float32, kind="ExternalInput").ap()
    out_d = nc.dram_tensor("out", [BPC, T, H], dt.float32, kind="ExternalOutput").ap()

    cpool = ctx.enter_context(tc.tile_pool(name="consts", bufs=1))
    xpool = ctx.enter_context(tc.tile_pool(name="x", bufs=18))
    wkpool = ctx.enter_context(tc.tile_pool(name="work", bufs=6))
    rpool = ctx.enter_context(tc.tile_pool(name="raw", bufs=10))
    qkpool = ctx.enter_context(tc.tile_pool(name="qk", bufs=10))
    vpool = ctx.enter_context(tc.tile_pool(name="v", bufs=25))
    apool = ctx.enter_context(tc.tile_pool(name="attn", bufs=12))
    spool = ctx.enter_context(tc.tile_pool(name="small", bufs=4))
    opool = ctx.enter_context(tc.tile_pool(name="outb", bufs=3))

    # PSUM (8 banks): {qkv, transpose, sigma-alpha} x2 + z/L x6 (1-bank slots)
    ps_sm = ctx.enter_context(tc.tile_pool(name="ps_sm", bufs=3, space="PSUM"))
    ps_zl = ctx.enter_context(tc.tile_pool(name="ps_zl", bufs=5, space="PSUM"))

    w_ext = cpool.tile([128, 6 * 256], dt.float32r)
    for j in range(6):
        nc.sync.dma_start(w_ext[:, j * 256:(j + 1) * 256], wext_d[j])
    w_sa = cpool.tile([64, 4], dt.float32r)
    nc.sync.dma_start(w_sa[:], wsa_d)
    dxy2s = cpool.tile([128, NCH * FW], dt.float32)
    nc.sync.dma_start(dxy2s[:], dxy_d)
    sel = cpool.tile([FW, NCH * 128], dt.float32r)
    nc.sync.dma_start(sel[:], sel_d)
    ident = cpool.tile([128, 128], dt.float32r)
    nc.sync.dma_start(ident[:], id_d)
    onz = cpool.tile([128, 2], dt.float32r)
    nc.sync.dma_start(onz[:], onz_d)
    gb = None
    if need_gb:
        gb = cpool.tile([4, 64], dt.float32)
        nc.sync.dma_start(gb[:], gb_d)

    _bias_cache = {}

    def fbias(val, tsz=128):
        val = float(val)
        if val == 0.0:
            return 0.0
        if val not in _bias_cache:
            bt = cpool.tile([128, 1], dt.float32, name=f"bias{len(_bias_cache)}")
            nc.vector.memset(bt[:], val)
            _bias_cache[val] = bt
        return _bias_cache[val][0:tsz, :]

    def phase1(S, b):
        """x DMA, QKV matmuls, raw evac, LN stats."""
        S["xt"] = xt = []
        for j in range(6):
            xj = xpool.tile([128, T], dt.float32r, tag="xT", name=f"x{b}_{j}")
            nc.sync.dma_start(xj[:], xT_d[b, j * 128:(j + 1) * 128, :])
            xt.append(xj)
        S["raw"] = raw = [
            rpool.tile([128, 192], dt.float32, tag="raw", name=f"raw{b}_{i}")
            for i in range(NCH)]
        S["mv"] = mv = spool.tile([128, 20], dt.float32, tag="mv", name=f"mv{b}")
        for c in range(NCH):
            tsz, toff = TSZ[c], TOFF[c]
            pqkv = ps_sm.tile([128, 256], dt.float32, tag="sm", name=f"qkv{b}_{c}")
            for j in range(6):
                nc.tensor.matmul(
                    pqkv[0:tsz, :], xt[j][:, toff:toff + tsz],
                    w_ext[:, j * 256:(j + 1) * 256],
                    start=(j == 0), stop=(j == 5),
                )
            nc.vector.tensor_copy(raw[c][0:tsz, :], pqkv[0:tsz, 0:192])
            st = spool.tile([128, 12], dt.float32, tag="st", name=f"st{b}_{c}")
            nc.vector.bn_stats(st[0:tsz, 0:6], raw[c][0:tsz, 0:64])
            nc.vector.bn_stats(st[0:tsz, 6:12], raw[c][0:tsz, 64:128])
            nc.vector.bn_aggr(mv[0:tsz, 4 * c:4 * c + 2], st[0:tsz, 0:6])
            nc.vector.bn_aggr(mv[0:tsz, 4 * c + 2:4 * c + 4], st[0:tsz, 6:12])

    def phase2(S, b):
        """LN params + apply, v build, q/k transposes into qkT."""
        mv = S["mv"]
        raw = S["raw"]
        mv4 = mv[:].rearrange("p (c f) -> p c f", f=4)
        lnv = spool.tile([128, 10], dt.float32, tag="lnv", name=f"lnv{b}")
        lnv2 = lnv[:].rearrange("p (c f) -> p c f", f=2)
        nc.scalar.activation(lnv2[:, :, :], mv4[:, :, 1::2], AF.Ln, bias=fbias(EPS))
        sc = spool.tile([128, 10], dt.float32, tag="sc", name=f"sc{b}")
        sc2 = sc[:].rearrange("p (c f) -> p c f", f=2)
        nc.scalar.activation(sc2[:, :, 0], lnv2[:, :, 0], AF.Exp,
                             bias=fbias(float(np.log(0.125))), scale=-0.5)
        nc.scalar.activation(sc2[:, :, 1], lnv2[:, :, 1], AF.Exp, scale=-0.5)
        nmr = spool.tile([128, 10], dt.float32, tag="nmr", name=f"nmr{b}")
        nmr2 = nmr[:].rearrange("p (c f) -> p c f", f=2)
        nc.vector.scalar_tensor_tensor(
            nmr2[:, :, :], mv4[:, :, 0::2], -1.0, sc2[:, :, :], OP.mult, OP.mult)

        S["qkT"] = qkT = wkpool.tile([64, 1156], dt.float32r, tag="qkT",
                                     name=f"qkT{b}")
        S["v_ext"] = v_ext = [
            vpool.tile([128, 66], dt.float32r, tag="vext", name=f"vext{b}_{i}")
            for i in range(NCH)]
        for c in range(NCH):
            tsz, toff = TSZ[c], TOFF[c]
            t2 = tsz + (tsz & 1)
            qk_ln = qkpool.tile([128, 128], dt.float32r, tag="qkln",
                                name=f"qkln{b}_{c}")
            if t2 != tsz:
                nc.vector.tensor_copy(
                    qk_ln[64:66, :], onz[64:66, 1:2].broadcast_to([2, 128]))
            nc.vector.tensor_scalar(
                qk_ln[0:tsz, 0:64], raw[c][0:tsz, 0:64],
                sc[0:tsz, 2 * c:2 * c + 1], nmr[0:tsz, 2 * c:2 * c + 1],
                OP.mult, OP.add)
            nc.vector.tensor_scalar(
                qk_ln[0:tsz, 64:128], raw[c][0:tsz, 64:128],
                sc[0:tsz, 2 * c + 1:2 * c + 2], nmr[0:tsz, 2 * c + 1:2 * c + 2],
                OP.mult, OP.add)
            if need_gb:
                nc.vector.tensor_mul(qk_ln[0:tsz, 0:64], qk_ln[0:tsz, 0:64],
                                     gb[0:1, :].partition_broadcast(tsz))
                nc.vector.tensor_add(qk_ln[0:tsz, 0:64], qk_ln[0:tsz, 0:64],
                                     gb[1:2, :].partition_broadcast(tsz))
                nc.vector.tensor_mul(qk_ln[0:tsz, 64:128], qk_ln[0:tsz, 64:128],
                                     gb[2:3, :].partition_broadcast(tsz))
                nc.vector.tensor_add(qk_ln[0:tsz, 64:128], qk_ln[0:tsz, 64:128],
                                     gb[3:4, :].partition_broadcast(tsz))
            nc.vector.tensor_scalar(
                v_ext[c][0:tsz, 0:64], raw[c][0:tsz, 128:192], 1.0, None, OP.mult)
            nc.vector.tensor_copy(v_ext[c][0:tsz, 64:66], onz[0:tsz, :])

            tpc = ps_sm.tile([128, 256], dt.float32r, tag="sm", name=f"tp{b}_{c}")
            nc.tensor.transpose(tpc[0:64, 0:t2], qk_ln[0:t2, 0:64],
                                ident[0:t2, 0:t2])
            nc.tensor.transpose(tpc[0:64, 128:128 + t2], qk_ln[0:t2, 64:128],
                                ident[0:t2, 0:t2])
            ncols = min(128, 578 - toff)
            dst = qkT[:].rearrange("p (g q) -> p g q", g=2)[:, :, toff:toff + ncols]
            nc.vector.tensor_copy(
                dst, tpc[0:64, :].rearrange("p (g q) -> p g q", g=2)[:, :, 0:ncols])

    def phase3(S, b):
        """sigma/alpha MLP, log-space factor build, factor transposes."""
        qkT = S["qkT"]
        psa = ps_sm.tile([128, 256], dt.float32, tag="sm", name=f"psa{b}")
        for c in range(NCH):
            nc.tensor.matmul(
                psa[0:TSZ[c], 4 * c:4 * c + 4], qkT[:, TOFF[c]:TOFF[c] + TSZ[c]],
                w_sa[:], start=True, stop=True)
        sap = spool.tile([128, 20], dt.float32, tag="sap", name=f"sap{b}")
        nc.vector.tensor_copy(sap[:], psa[:, 0:20])
        sap4 = sap[:].rearrange("p (c f) -> p c f", f=4)
        texp = spool.tile([128, 10], dt.float32, tag="texp", name=f"texp{b}")
        texp3 = texp[:].rearrange("p (c f) -> p c f", f=2)
        if bs0 == bs1:
            nc.scalar.activation(texp3[:, :, :], sap4[:, :, 0:2], AF.Exp,
                                 bias=fbias(-bs0), scale=-1.0)
        else:
            for col in range(2):
                nc.scalar.activation(texp3[:, :, col], sap4[:, :, col], AF.Exp,
                                     bias=fbias(-(bs0 if col == 0 else bs1)),
                                     scale=-1.0)
        ab = spool.tile([128, 10], dt.float32, tag="ab", name=f"ab{b}")
        nc.vector.tensor_scalar_add(ab[:], texp[:], 1.0)
        nc.vector.tensor_mul(ab[:], ab[:], ab[:])
        spe = spool.tile([128, 5], dt.float32, tag="spe", name=f"spe{b}")
        nc.scalar.activation(spe[:], sap4[:, :, 2], AF.Exp, bias=fbias(ba0))
        spl = spool.tile([128, 5], dt.float32, tag="spl", name=f"spl{b}")
        nc.scalar.activation(spl[:], spe[:], AF.Ln, bias=fbias(1.0))
        lna = spool.tile([128, 5], dt.float32, tag="lna", name=f"lna{b}")
        nc.scalar.activation(lna[:], spl[:], AF.Ln, scale=0.125)

        yn = spool.tile([128, NCH * FW], dt.float32r, tag="yn", name=f"yn{b}")
        dxy3 = dxy2s[:].rearrange("p (c f) -> p c f", f=FW)
        yn3 = yn[:].rearrange("p (c f) -> p c f", f=FW)
        ab3 = ab[:].rearrange("p (c f) -> p c f", f=2)
        nc.vector.scalar_tensor_tensor(
            yn3[:, :, 0:24], dxy3[:, :, 0:24], 1.0,
            ab3[:, :, 1:2].broadcast_to([128, NCH, 24]), OP.mult, OP.mult)
        nc.vector.scalar_tensor_tensor(
            yn3[:, :, 25:50], dxy3[:, :, 25:50], 1.0,
            ab3[:, :, 0:1].broadcast_to([128, NCH, 25]), OP.mult, OP.mult)
        nc.vector.tensor_copy(yn3[:, :, 24:25], lna[:].unsqueeze(-1))

        S["yT"] = yT = wkpool.tile([FW, 578], dt.float32r, tag="yT", name=f"yT{b}")
        for c in range(NCH):
            t2 = TSZ[c] + (TSZ[c] & 1)
            pfc = ps_sm.tile([128, 256], dt.float32r, tag="sm", name=f"pf{b}_{c}")
            nc.tensor.transpose(pfc[0:FW, 0:t2], yn[0:t2, c * FW:(c + 1) * FW],
                                ident[0:t2, 0:t2])
            nc.vector.tensor_copy(yT[:, TOFF[c]:TOFF[c] + t2], pfc[0:FW, 0:t2])

    def phase4(S, b):
        """main loop: sim^T + bias + exp per k-chunk (transposed softmax)."""
        qkT, yT = S["qkT"], S["yT"]
        S["attnT"] = attnT = [
            apool.tile([128, T], dt.float32r, tag="attnT", name=f"attnT{b}_{i}")
            for i in range(NCH)]
        for c in range(NCH):
            tsz, toff = TSZ[c], TOFF[c]
            selc = sel[:, c * 128:c * 128 + tsz]
            kTc = qkT[:, 578 + toff:578 + toff + tsz]
            expl = wkpool.tile([128, 578], dt.float32r, tag="expl",
                               name=f"expl{b}_{c}")
            pla = ps_zl.tile([128, 320], dt.float32, tag="zl", name=f"pl{b}{c}a")
            plb = ps_zl.tile([128, 320], dt.float32, tag="zl", name=f"pl{b}{c}b")
            nc.tensor.matmul(pla[0:tsz, :], selc, yT[:, 0:320],
                             start=True, stop=True)
            nc.tensor.matmul(plb[0:tsz, 0:258], selc, yT[:, 320:578],
                             start=True, stop=True)
            nc.scalar.activation(expl[0:tsz, 0:320], pla[0:tsz, :], AF.Exp)
            nc.scalar.activation(expl[0:tsz, 320:578], plb[0:tsz, 0:258], AF.Exp)
            pza = ps_zl.tile([128, 320], dt.float32, tag="zl", name=f"pz{b}{c}a")
            pzb = ps_zl.tile([128, 320], dt.float32, tag="zl", name=f"pz{b}{c}b")
            nc.tensor.matmul(pza[0:tsz, :], kTc, qkT[:, 0:320],
                             start=True, stop=False)
            nc.tensor.matmul(pzb[0:tsz, 0:258], kTc, qkT[:, 320:578],
                             start=True, stop=False)
            nc.tensor.matmul(pza[0:tsz, :], ident[0:tsz, 0:tsz],
                             expl[0:tsz, 0:320], start=False, stop=True)
            nc.tensor.matmul(pzb[0:tsz, 0:258], ident[0:tsz, 0:tsz],
                             expl[0:tsz, 320:578], start=False, stop=True)
            nc.scalar.activation(attnT[c][0:tsz, 0:320], pza[0:tsz, :], AF.Exp)
            nc.scalar.activation(attnT[c][0:tsz, 320:T], pzb[0:tsz, 0:257], AF.Exp)

    def phase5(S, b):
        """attn @ [v|1] + batched normalize + store."""
        attnT, v_ext = S["attnT"], S["v_ext"]
        osb = opool.tile([128, 320], dt.float32, tag="osb", name=f"osb{b}")
        for g, qcs in enumerate(((0, 1), (2, 3), (4,))):
            po = ps_zl.tile([128, 256], dt.float32, tag="zl", name=f"po{b}_{g}")
            for i, qc in enumerate(qcs):
                qsz, qoff = TSZ[qc], TOFF[qc]
                for kc in range(NCH):
                    nc.tensor.matmul(
                        po[0:qsz, 128 * i:128 * i + 66],
                        attnT[kc][0:TSZ[kc], qoff:qoff + qsz],
                        v_ext[kc][0:TSZ[kc], :], start=(kc == 0), stop=(kc == 4))
            n = len(qcs)
            po5 = po[:].rearrange("p (c f) -> p c f", f=128)[:, 0:n, :]
            rcp = spool.tile([128, 2], dt.float32, tag="rcp", name=f"rcp{b}_{g}")
            nc.vector.reciprocal(rcp[:, 0:n], po5[:, :, 64])
            osb3 = osb[:].rearrange("p (c f) -> p c f", f=64)[:, 2 * g:2 * g + n, :]
            nc.vector.scalar_tensor_tensor(
                osb3, po5[:, :, 0:64], 1.0,
                rcp[:, 0:n].unsqueeze(-1).broadcast_to([128, n, 64]),
                OP.mult, OP.mult)
        nc.sync.dma_start(
            out_d[b, 0:512, :].rearrange("(c p) h -> p c h", p=128),
            osb[:, 0:256].rearrange("p (c h) -> p c h", h=64))
        nc.sync.dma_start(out_d[b, 512:T, :], osb[0:65, 256:320])

    # anti-diagonal software pipeline across batches
    phases = [phase1, phase2, phase3, phase4, phase5]
    states = [dict() for _ in range(BPC)]
    for slot in range(BPC + NPH - 1):
        for p in range(NPH - 1, -1, -1):
            b = slot - p
            if 0 <= b < BPC:
                phases[p](states[b], b)


_CACHE = {}


def _build(consts_f, need_gb):
    import concourse.tile as tile
    from concourse import bacc

    key = (consts_f, need_gb)
    if key in _CACHE:
        return _CACHE[key]
    nc = bacc.Bacc("TRN2", target_bir_lowering=False, debug=False)
    with tile.TileContext(nc) as tc, ExitStack() as ctx:
        _trace(nc, tc, ctx, consts_f, need_gb)
    nc.finalize()
    _CACHE[key] = nc
    return nc


def kernel(x, w_q, w_k, w_v, q_gamma, q_beta, k_gamma, k_beta,
           w_sigma, b_sigma, w_alpha, b_alpha):
    from concourse import bass_utils

    x = np.asarray(x, np.float32)
    w_q, w_k, w_v = (np.asarray(a, np.float32) for a in (w_q, w_k, w_v))
    w_sigma = np.asarray(w_sigma, np.float32)
    w_alpha = np.asarray(w_alpha, np.float32)
    b_sigma = np.asarray(b_sigma, np.float32)
    b_alpha = np.asarray(b_alpha, np.float32)
    q_gamma, q_beta = np.asarray(q_gamma, np.float32), np.asarray(q_beta, np.float32)
    k_gamma, k_beta = np.asarray(k_gamma, np.float32), np.asarray(k_beta, np.float32)

    trivial_gb = (
        np.allclose(q_gamma, 1) and np.allclose(k_gamma, 1)
        and np.allclose(q_beta, 0) and np.allclose(k_beta, 0)
    )

    w_ext, w_sa, dxy2s, sel, ident, onz = _host_consts(
        w_q, w_k, w_v, w_sigma, w_alpha)
    consts_f = (float(b_sigma[0]), float(b_sigma[1]), float(b_alpha[0]))
    nc = _build(consts_f, not trivial_gb)

    xt = np.ascontiguousarray(x.reshape(NCORES, BPC, T, E).transpose(0, 1, 3, 2))

    base = {
        "w_ext": w_ext, "w_sa": w_sa, "dxy2s": dxy2s, "sel": sel, "ident": ident,
        "onz": onz,
    }
    if not trivial_gb:
        base["gb"] = np.stack(
            [q_gamma, q_beta / 8.0, k_gamma, k_beta]).astype(np.float32)
    in_maps = [{**base, "xT": xt[c]} for c in range(NCORES)]

    res = bass_utils.run_bass_kernel_spmd(nc, in_maps, core_ids=list(range(NCORES)))
    out = np.concatenate([res.results[c]["out"] for c in range(NCORES)], axis=0)
    return out.astype(np.float32)


# revision 27
# speedup vs baseline: 1.3803x; 1.2467x over previous
"""Trainium2 Bass kernel for nn_AugmentedAttentionHead.

See kernel design notes: data-parallel over batch (8/core); transposed
softmax ([k-part, q-free]) so exp() output feeds attn@v directly as the
stationary operand; log-space Gaussian bias reconstructed by one selector
matmul per k-chunk; x host-pre-transposed; f32r matmuls; ln/exp-only ACT
table; prefix kills folded into host tables.

The per-batch work is split into 5 phases and traced along anti-diagonals
(software pipelining across batches) so each engine always has another
batch's independent work to fill dependency stalls.
"""

import numpy as np
from contextlib import ExitStack

B, T, E, H = 64, 577, 768, 64
GRID = 24
EPS = 1e-5
NCORES = 8
BPC = B // NCORES

TSZ = [128, 128, 128, 128, 65]
TOFF = [0, 128, 256, 384, 512]
NCH = 5
FW = 50
NEG = -1.0e30
NPH = 5


def _host_consts(w_q, w_k, w_v, w_sigma, w_alpha):
    w_ext = np.zeros((6, 128, 256), np.float32)
    wqkv = np.concatenate([w_q, w_k, w_v], axis=1)
    for j in range(6):
        w_ext[j, :, :192] = wqkv[j * 128:(j + 1) * 128]

    w_sa = np.zeros((64, 4), np.float32)
    w_sa[:, 0:2] = 8.0 * w_sigma
    w_sa[:, 2] = 8.0 * w_alpha[:, 0]

    dxy2s = np.zeros((128, NCH * FW), np.float32)
    for c in range(NCH):
        for r in range(TSZ[c]):
            t = TOFF[c] + r
            o = c * FW
            dxy2s[r, o + 49] = NEG
            if t == 0:
                dxy2s[r, o:o + 24] = NEG
                dxy2s[r, o + 25:o + 49] = NEG
                continue
            p = t - 1
            py, px = p // GRID, p % GRID
            j = np.arange(GRID, dtype=np.float32)
            dxy2s[r, o:o + 24] = -0.5 * (py - j) ** 2
            dxy2s[r, o + 25:o + 49] = -0.5 * (px - j) ** 2

    sel = np.zeros((FW, NCH * 128), np.float32)
    for c in range(NCH):
        for r in range(TSZ[c]):
            t = TOFF[c] + r
            if t == 0:
                sel[49, c * 128 + r] = 1.0
                continue
            p = t - 1
            sel[p // GRID, c * 128 + r] = 1.0
            sel[24, c * 128 + r] = 1.0
            sel[25 + p % GRID, c * 128 + r] = 1.0

    ident = np.eye(128, dtype=np.float32)
    onz = np.stack([np.ones(128), np.zeros(128)], 1).astype(np.float32)
    return w_ext, w_sa, dxy2s, sel, ident, onz


def _trace(nc, tc, ctx, consts_f, need_gb):
    import concourse.mybir as mybir

    dt = mybir.dt
    AF = mybir.ActivationFunctionType
    OP = mybir.AluOpType
    bs0, bs1, ba0 = consts_f[:3]

    xT_d = nc.dram_tensor("xT", [BPC, E, T], dt.float32r, kind="ExternalInput").ap()
    wext_d = nc.dram_tensor("w_ext", [6, 128, 256], dt.float32r, kind="ExternalInput").ap()
    wsa_d = nc.dram_tensor("w_sa", [64, 4], dt.float32r, kind="ExternalInput").ap()
    dxy_d = nc.dram_tensor("dxy2s", [128, NCH * FW], dt.float32, kind="ExternalInput").ap()
    sel_d = nc.dram_tensor("sel", [FW, NCH * 128], dt.float32r, kind="ExternalInput").ap()
    id_d = nc.dram_tensor("ident", [128, 128], dt.float32r, kind="ExternalInput").ap()
    onz_d = nc.dram_tensor("onz", [128, 2], dt.float32r, kind="ExternalInput").ap()
    if need_gb:
        gb_d = nc.dram_tensor("gb", [4, 64], dt.float32, kind="ExternalInput").ap()
    out_d = nc.dram_tensor("out", [BPC, T, H], dt.float32, kind="ExternalOutput").ap()

    cpool = ctx.enter_context(tc.tile_pool(name="consts", bufs=1))
    xpool = ctx.enter_context(tc.tile_pool(name="x", bufs=18))
    wkpool = ctx.enter_context(tc.tile_pool(name="work", bufs=6))
    rpool = ctx.enter_context(tc.tile_pool(name="raw", bufs=10))
    qkpool = ctx.enter_context(tc.tile_pool(name="qk", bufs=10))
    vpool = ctx.enter_context(tc.tile_pool(name="v", bufs=25))
    apool = ctx.enter_context(tc.tile_pool(name="attn", bufs=12))
    spool = ctx.enter_context(tc.tile_pool(name="small", bufs=4))
    opool = ctx.enter_context(tc.tile_pool(name="outb", bufs=3))

    # PSUM (8 banks): {qkv, transpose, sigma-alpha} x2 + z/L x6 (1-bank slots)
    ps_sm = ctx.enter_context(tc.tile_pool(name="ps_sm", bufs=3, space="PSUM"))
    ps_zl = ctx.enter_context(tc.tile_pool(name="ps_zl", bufs=5, space="PSUM"))

    w_ext = cpool.tile([128, 6 * 256], dt.float32r)
    for j in range(6):
        nc.sync.dma_start(w_ext[:, j * 256:(j + 1) * 256], wext_d[j])
    w_sa = cpool.tile([64, 4], dt.float32r)
    nc.sync.dma_start(w_sa[:], wsa_d)
    dxy2s = cpool.tile([128, NCH * FW], dt.float32)
    nc.sync.dma_start(dxy2s[:], dxy_d)
    sel = cpool.tile([FW, NCH * 128], dt.float32r)
    nc.sync.dma_start(sel[:], sel_d)
    ident = cpool.tile([128, 128], dt.float32r)
    nc.sync.dma_start(ident[:], id_d)
    onz = cpool.tile([128, 2], dt.float32r)
    nc.sync.dma_start(onz[:], onz_d)
    gb = None
    if need_gb:
        gb = cpool.tile([4, 64], dt.float32)
        nc.sync.dma_start(gb[:], gb_d)

    _bias_cache = {}

    def fbias(val, tsz=128):
        val = float(val)
        if val == 0.0:
            return 0.0
        if val not in _bias_cache:
            bt = cpool.tile([128, 1], dt.float32, name=f"bias{len(_bias_cache)}")
            nc.vector.memset(bt[:], val)
            _bias_cache[val] = bt
        return _bias_cache[val][0:tsz, :]

    def phase1(S, b):
        """x DMA, QKV matmuls, raw evac, LN stats."""
        S["xt"] = xt = []
        for j in range(6):
            xj = xpool.tile([128, T], dt.float32r, tag="xT", name=f"x{b}_{j}")
            nc.sync.dma_start(xj[:], xT_d[b, j * 128:(j + 1) * 128, :])
            xt.append(xj)
        S["raw"] = raw = [
            rpool.tile([128, 192], dt.float32, tag="raw", name=f"raw{b}_{i}")
            for i in range(NCH)]
        S["mv"] = mv = spool.tile([128, 20], dt.float32, tag="mv", name=f"mv{b}")
        for c in range(NCH):
            tsz, toff = TSZ[c], TOFF[c]
            pqkv = ps_sm.tile([128, 256], dt.float32, tag="sm", name=f"qkv{b}_{c}")
            for j in range(6):
                nc.tensor.matmul(
                    pqkv[0:tsz, :], xt[j][:, toff:toff + tsz],
                    w_ext[:, j * 256:(j + 1) * 256],
                    start=(j == 0), stop=(j == 5),
                )
            nc.vector.tensor_copy(raw[c][0:tsz, :], pqkv[0:tsz, 0:192])
            st = spool.tile([128, 12], dt.float32, tag="st", name=f"st{b}_{c}")
            nc.vector.bn_stats(st[0:tsz, 0:6], raw[c][0:tsz, 0:64])
            nc.vector.bn_stats(st[0:tsz, 6:12], raw[c][0:tsz, 64:128])
            nc.vector.bn_aggr(mv[0:tsz, 4 * c:4 * c + 2], st[0:tsz, 0:6])
            nc.vector.bn_aggr(mv[0:tsz, 4 * c + 2:4 * c + 4], st[0:tsz, 6:12])

    def phase2(S, b):
        """LN params + apply, v build, q/k transposes into qkT."""
        mv = S["mv"]
        raw = S["raw"]
        mv4 = mv[:].rearrange("p (c f) -> p c f", f=4)
        lnv = spool.tile([128, 10], dt.float32, tag="lnv", name=f"lnv{b}")
        lnv2 = lnv[:].rearrange("p (c f) -> p c f", f=2)
        nc.scalar.activation(lnv2[:, :, :], mv4[:, :, 1::2], AF.Ln, bias=fbias(EPS))
        sc = spool.tile([128, 10], dt.float32, tag="sc", name=f"sc{b}")
        sc2 = sc[:].rearrange("p (c f) -> p c f", f=2)
        nc.scalar.activation(sc2[:, :, 0], lnv2[:, :, 0], AF.Exp,
                             bias=fbias(float(np.log(0.125))), scale=-0.5)
        nc.scalar.activation(sc2[:, :, 1], lnv2[:, :, 1], AF.Exp, scale=-0.5)
        nmr = spool.tile([128, 10], dt.float32, tag="nmr", name=f"nmr{b}")
        nmr2 = nmr[:].rearrange("p (c f) -> p c f", f=2)
        nc.vector.scalar_tensor_tensor(
            nmr2[:, :, :], mv4[:, :, 0::2], -1.0, sc2[:, :, :], OP.mult, OP.mult)

        S["qkT"] = qkT = wkpool.tile([64, 1156], dt.float32r, tag="qkT",
                                     name=f"qkT{b}")
        S["v_ext"] = v_ext = [
            vpool.tile([128, 66], dt.float32r, tag="vext", name=f"vext{b}_{i}")
            for i in range(NCH)]
        for c in range(NCH):
            tsz, toff = TSZ[c], TOFF[c]
            t2 = tsz + (tsz & 1)
            qk_ln = qkpool.tile([128, 128], dt.float32r, tag="qkln",
                                name=f"qkln{b}_{c}")
            if t2 != tsz:
                nc.vector.tensor_copy(
                    qk_ln[64:66, :], onz[64:66, 1:2].broadcast_to([2, 128]))
            nc.vector.tensor_scalar(
                qk_ln[0:tsz, 0:64], raw[c][0:tsz, 0:64],
                sc[0:tsz, 2 * c:2 * c + 1], nmr[0:tsz, 2 * c:2 * c + 1],
                OP.mult, OP.add)
            nc.vector.tensor_scalar(
                qk_ln[0:tsz, 64:128], raw[c][0:tsz, 64:128],
                sc[0:tsz, 2 * c + 1:2 * c + 2], nmr[0:tsz, 2 * c + 1:2 * c + 2],
                OP.mult, OP.add)
            if need_gb:
                nc.vector.tensor_mul(qk_ln[0:tsz, 0:64], qk_ln[0:tsz, 0:64],
                                     gb[0:1, :].partition_broadcast(tsz))
                nc.vector.tensor_add(qk_ln[0:tsz, 0:64], qk_ln[0:tsz, 0:64],
                                     gb[1:2, :].partition_broadcast(tsz))
                nc.vector.tensor_mul(qk_ln[0:tsz, 64:128], qk_ln[0:tsz, 64:128],
                                     gb[2:3, :].partition_broadcast(tsz))
                nc.vector.tensor_add(qk_ln[0:tsz, 64:128], qk_ln[0:tsz, 64:128],
                                     gb[3:4, :].partition_broadcast(tsz))
            nc.vector.tensor_scalar(
                v_ext[c][0:tsz, 0:64], raw[c][0:tsz, 128:192], 1.0, None, OP.mult)
            nc.vector.tensor_copy(v_ext[c][0:tsz, 64:66], onz[0:tsz, :])

            tpc = ps_sm.tile([128, 256], dt.float32r, tag="sm", name=f"tp{b}_{c}")
            nc.tensor.transpose(tpc[0:64, 0:t2], qk_ln[0:t2, 0:64],
                                ident[0:t2, 0:t2])
            nc.tensor.transpose(tpc[0:64, 128:128 + t2], qk_ln[0:t2, 64:128],
                                ident[0:t2, 0:t2])
            ncols = min(128, 578 - toff)
            dst = qkT[:].rearrange("p (g q) -> p g q", g=2)[:, :, toff:toff + ncols]
            nc.vector.tensor_copy(
                dst, tpc[0:64, :].rearrange("p (g q) -> p g q", g=2)[:, :, 0:ncols])

    def phase3(S, b):
        """sigma/alpha MLP, log-space factor build, factor transposes."""
        qkT = S["qkT"]
        psa = ps_sm.tile([128, 256], dt.float32, tag="sm", name=f"psa{b}")
        for c in range(NCH):
            nc.tensor.matmul(
                psa[0:TSZ[c], 4 * c:4 * c + 4], qkT[:, TOFF[c]:TOFF[c] + TSZ[c]],
                w_sa[:], start=True, stop=True)
        sap = spool.tile([128, 20], dt.float32, tag="sap", name=f"sap{b}")
        nc.vector.tensor_copy(sap[:], psa[:, 0:20])
        sap4 = sap[:].rearrange("p (c f) -> p c f", f=4)
        texp = spool.tile([128, 10], dt.float32, tag="texp", name=f"texp{b}")
        texp3 = texp[:].rearrange("p (c f) -> p c f", f=2)
        if bs0 == bs1:
            nc.scalar.activation(texp3[:, :, :], sap4[:, :, 0:2], AF.Exp,
                                 bias=fbias(-bs0), scale=-1.0)
        else:
            for col in range(2):
                nc.scalar.activation(texp3[:, :, col], sap4[:, :, col], AF.Exp,
                                     bias=fbias(-(bs0 if col == 0 else bs1)),
                                     scale=-1.0)
        ab = spool.tile([128, 10], dt.float32, tag="ab", name=f"ab{b}")
        nc.vector.tensor_scalar_add(ab[:], texp[:], 1.0)
        nc.vector.tensor_mul(ab[:], ab[:], ab[:])
        spe = spool.tile([128, 5], dt.float32, tag="spe", name=f"spe{b}")
        nc.scalar.activation(spe[:], sap4[:, :, 2], AF.Exp, bias=fbias(ba0))
        spl = spool.tile([128, 5], dt.float32, tag="spl", name=f"spl{b}")
        nc.scalar.activation(spl[:], spe[:], AF.Ln, bias=fbias(1.0))
        lna = spool.tile([128, 5], dt.float32, tag="lna", name=f"lna{b}")
        nc.scalar.activation(lna[:], spl[:], AF.Ln, scale=0.125)

        yn = spool.tile([128, NCH * FW], dt.float32r, tag="yn", name=f"yn{b}")
        dxy3 = dxy2s[:].rearrange("p (c f) -> p c f", f=FW)
        yn3 = yn[:].rearrange("p (c f) -> p c f", f=FW)
        ab3 = ab[:].rearrange("p (c f) -> p c f", f=2)
        nc.vector.scalar_tensor_tensor(
            yn3[:, :, 0:24], dxy3[:, :, 0:24], 1.0,
            ab3[:, :, 1:2].broadcast_to([128, NCH, 24]), OP.mult, OP.mult)
        nc.vector.scalar_tensor_tensor(
            yn3[:, :, 25:50], dxy3[:, :, 25:50], 1.0,
            ab3[:, :, 0:1].broadcast_to([128, NCH, 25]), OP.mult, OP.mult)
        nc.vector.tensor_copy(yn3[:, :, 24:25], lna[:].unsqueeze(-1))

        S["yT"] = yT = wkpool.tile([FW, 578], dt.float32r, tag="yT", name=f"yT{b}")
        for c in range(NCH):
            t2 = TSZ[c] + (TSZ[c] & 1)
            pfc = ps_sm.tile([128, 256], dt.float32r, tag="sm", name=f"pf{b}_{c}")
            nc.tensor.transpose(pfc[0:FW, 0:t2], yn[0:t2, c * FW:(c + 1) * FW],
                                ident[0:t2, 0:t2])
            nc.vector.tensor_copy(yT[:, TOFF[c]:TOFF[c] + t2], pfc[0:FW, 0:t2])

    def phase4(S, b):
        """main loop: sim^T + bias + exp per k-chunk (transposed softmax)."""
        qkT, yT = S["qkT"], S["yT"]
        S["attnT"] = attnT = [
            apool.tile([128, T], dt.float32r, tag="attnT", name=f"attnT{b}_{i}")
            for i in range(NCH)]
        for c in range(NCH):
            tsz, toff = TSZ[c], TOFF[c]
            selc = sel[:, c * 128:c * 128 + tsz]
            kTc = qkT[:, 578 + toff:578 + toff + tsz]
            expl = wkpool.tile([128, 578], dt.float32r, tag="expl",
                               name=f"expl{b}_{c}")
            pla = ps_zl.tile([128, 320], dt.float32, tag="zl", name=f"pl{b}{c}a")
            plb = ps_zl.tile([128, 320], dt.float32, tag="zl", name=f"pl{b}{c}b")
            nc.tensor.matmul(pla[0:tsz, :], selc, yT[:, 0:320],
                             start=True, stop=True)
            nc.tensor.matmul(plb[0:tsz, 0:258], selc, yT[:, 320:578],
                             start=True, stop=True)
            nc.scalar.activation(expl[0:tsz, 0:320], pla[0:tsz, :], AF.Exp)
            nc.scalar.activation(expl[0:tsz, 320:578], plb[0:tsz, 0:258], AF.Exp)
            pza = ps_zl.tile([128, 320], dt.float32, tag="zl", name=f"pz{b}{c}a")
            pzb = ps_zl.tile([128, 320], dt.float32, tag="zl", name=f"pz{b}{c}b")
            nc.tensor.matmul(pza[0:tsz, :], kTc, qkT[:, 0:320],
                             start=True, stop=False)
            nc.tensor.matmul(pzb[0:tsz, 0:258], kTc, qkT[:, 320:578],
                             start=True, stop=False)
            nc.tensor.matmul(pza[0:tsz, :], ident[0:tsz, 0:tsz],
                             expl[0:tsz, 0:320], start=False, stop=True)
            nc.tensor.matmul(pzb[0:tsz, 0:258], ident[0:tsz, 0:tsz],
                             expl[0:tsz, 320:578], start=False, stop=True)
            nc.scalar.activation(attnT[c][0:tsz, 0:320], pza[0:tsz, :], AF.Exp)
            nc.scalar.activation(attnT[c][0:tsz, 320:T], pzb[0:tsz, 0:257], AF.Exp)

    def phase5(S, b):
        """attn @ [v|1] + batched normalize + store."""
        attnT, v_ext = S["attnT"], S["v_ext"]
        osb = opool.tile([128, 320], dt.float32, tag="osb", name=f"osb{b}")
        for g, qcs in enumerate(((0, 1), (2, 3), (4,))):
            po = ps_zl.tile([128, 256], dt.float32, tag="zl", name=f"po{b}_{g}")
            for i, qc in enumerate(qcs):
                qsz, qoff = TSZ[qc], TOFF[qc]
                for kc in range(NCH):
                    nc.tensor.matmul(
                        po[0:qsz, 128 * i:128 * i + 66],
                        attnT[kc][0:TSZ[kc], qoff:qoff + qsz],
                        v_ext[kc][0:TSZ[kc], :], start=(kc == 0), stop=(kc == 4))
            n = len(qcs)
            po5 = po[:].rearrange("p (c f) -> p c f", f=128)[:, 0:n, :]
            rcp = spool.tile([128, 2], dt.float32, tag="rcp", name=f"rcp{b}_{g}")
            nc.vector.reciprocal(rcp[:, 0:n], po5[:, :, 64])
            osb3 = osb[:].rearrange("p (c f) -> p c f", f=64)[:, 2 * g:2 * g + n, :]
            nc.vector.scalar_tensor_tensor(
                osb3, po5[:, :, 0:64], 1.0,
                rcp[:, 0:n].unsqueeze(-1).broadcast_to([128, n, 64]),
                OP.mult, OP.mult)
        nc.sync.dma_start(
            out_d[b, 0:512, :].rearrange("(c p) h -> p c h", p=128),
            osb[:, 0:256].rearrange("p (c h) -> p c h", h=64))
        nc.sync.dma_start(out_d[b, 512:T, :], osb[0:65, 256:320])

    # anti-diagonal software pipeline across batches
    phases = [phase1, phase2, phase3, phase4, phase5]
    states = [dict() for _ in range(BPC)]
    for slot in range(BPC + NPH - 1):
        for p in range(NPH - 1, -1, -1):
            b = slot - p
            if 0 <= b < BPC:
                phases[p](states[b], b)


_CACHE = {}


def _patch_act_tables():
    # bacc's insert_act_table_loads maps each activation func to the first
    # table containing it, which makes Exp<->Ln transitions reload tables
    # (1.28 us each, ~30x per kernel). Restrict the funcs this kernel uses
    # to the combined natural_log_exp_and_others set so one load suffices.
    import concourse.bacc as bacc_mod
    import concourse.mybir as mybir
    from concourse.hw_specs import get_activation_tables as _gat
    if getattr(bacc_mod, "_ant_act_tables_patched", False):
        return
    AF = mybir.ActivationFunctionType
    mine = {AF.Exp, AF.Ln, AF.Copy, AF.Identity, AF.MemsetZero}

    def patched(arch):
        tabs = _gat(arch)
        combined = tabs.get("natural_log_exp_and_others")
        if combined and mine <= combined:
            for name, s in tabs.items():
                if name != "natural_log_exp_and_others":
                    tabs[name] = s - mine
        return tabs

    bacc_mod.get_activation_tables = patched
    bacc_mod._ant_act_tables_patched = True


def _build(consts_f, need_gb):
    import concourse.tile as tile
    from concourse import bacc

    _patch_act_tables()
    key = (consts_f, need_gb)
    if key in _CACHE:
        return _CACHE[key]
    nc = bacc.Bacc("TRN2", target_bir_lowering=False, debug=False)
    with tile.TileContext(nc) as tc, ExitStack() as ctx:
        _trace(nc, tc, ctx, consts_f, need_gb)
    nc.finalize()
    _CACHE[key] = nc
    return nc


def kernel(x, w_q, w_k, w_v, q_gamma, q_beta, k_gamma, k_beta,
           w_sigma, b_sigma, w_alpha, b_alpha):
    from concourse import bass_utils

    x = np.asarray(x, np.float32)
    w_q, w_k, w_v = (np.asarray(a, np.float32) for a in (w_q, w_k, w_v))
    w_sigma = np.asarray(w_sigma, np.float32)
    w_alpha = np.asarray(w_alpha, np.float32)
    b_sigma = np.asarray(b_sigma, np.float32)
    b_alpha = np.asarray(b_alpha, np.float32)
    q_gamma, q_beta = np.asarray(q_gamma, np.float32), np.asarray(q_beta, np.float32)
    k_gamma, k_beta = np.asarray(k_gamma, np.float32), np.asarray(k_beta, np.float32)

    trivial_gb = (
        np.allclose(q_gamma, 1) and np.allclose(k_gamma, 1)
        and np.allclose(q_beta, 0) and np.allclose(k_beta, 0)
    )

    w_ext, w_sa, dxy2s, sel, ident, onz = _host_consts(
        w_q, w_k, w_v, w_sigma, w_alpha)
    consts_f = (float(b_sigma[0]), float(b_sigma[1]), float(b_alpha[0]))
    nc = _build(consts_f, not trivial_gb)

    xt = np.ascontiguousarray(x.reshape(NCORES, BPC, T, E).transpose(0, 1, 3, 2))

    base = {
        "w_ext": w_ext, "w_sa": w_sa, "dxy2s": dxy2s, "sel": sel, "ident": ident,
        "onz": onz,
    }
    if not trivial_gb:
        base["gb"] = np.stack(
            [q_gamma, q_beta / 8.0, k_gamma, k_beta]).astype(np.float32)
    in_maps = [{**base, "xT": xt[c]} for c in range(NCORES)]

    res = bass_utils.run_bass_kernel_spmd(nc, in_maps, core_ids=list(range(NCORES)))
    out = np.concatenate([res.results[c]["out"] for c in range(NCORES)], axis=0)
    return out.astype(np.float32)
